# revision 19
# baseline (speedup 1.0000x reference)
"""Trainium2 Bass kernel for nn_AE_spikes (spiking autoencoder, 784-128-128-128-784).

Algorithm restructure (mathematically equivalent to the reference spiking net):
- Identity bin scaling (all 5 bin arrays equal) -> weights used as-is.
- Input layer digitize + integrate-and-fire has a closed form for the
  cumulative spike count: F_k = floor(k*a - 1/16), a = max(floor(16 f), 1)/16
  (m=0 and m=1 produce identical all-zero spike trains, so the clamp is exact).
- Each layer's matmul consumes the CUMULATIVE spike counts C of the previous
  layer, giving the cumulative drive D_k = W @ C_k directly (no per-step
  matmuls, no cumsum pass). One N=512 matmul per layer (K-chunked for the
  784-dim layers).
- The integrate-and-fire recurrence C_k = C_{k-1} + 1{t0 + D_k - C_{k-1} > 1}
  is computed as the max-chase C_k = max(C_{k-1}, ceil(b - 1 + D_k)), which is
  exact whenever the pre-fire potential stays <= 2 (holds for this model's
  weight scale; validated elementwise against the exact recurrence).
- ceil via the fp32 magic-round trick: round(x) = (x + 1.5*2^23) - 1.5*2^23,
  with the +0.5-delta ceil shift folded into the ACT bias.
- The whole 16-step recurrence of a layer runs as ONE DVE tensor_tensor_scan:
  state = max(Gceil_t, state) * mask_t, with 17-slot chains (16 steps + one
  masked dummy slot that resets the state to 0 between independent chains).

Sharding: pure data-parallel over the batch (256 -> 32 images per core), all
weights replicated, no collectives. Host pre-transposes weights/features so
every DMA is partition-contiguous, and reassembles the output.

Performance structure:
- fp16 matmul operands (full PE rate; counts are small integers, exact in f16).
- k-major F layout so the 16 closed-form input ops are contiguous f16 writes
  (DVE 4x mode); the layer-0 matmul reads (j,k) order via a strided rhs AP.
- Hidden layers run as two batch lanes (16 images each) pipelined across
  PE (matmul) / ACT (bias + psum read) / DVE (round + chase scan).
- Output layer needs no scan at all: its chase unrolls to
  relu(ceil(max_k G_k)) = one DVE max-reduce per 112-row chunk.
- PE warmup matmuls during the input phase so real matmuls run at 2.4 GHz.
"""
import sys

if "/opt/trn_rl_repo" not in sys.path:
    sys.path.insert(0, "/opt/trn_rl_repo")

import numpy as np

IN, HID, NS, NB = 784, 128, 16, 32  # in-dim, hidden, steps, batch per core
PCH, NCH = 112, 7                   # pixel-partition chunking: 784 = 112 * 7
SLOT = NS + 1                       # 17-slot chains (dummy slot resets scan state)
NCORES = 8
M32 = 12582912.0                    # 1.5 * 2^23: fp32 round-to-integer magic
DELTA = 2.0 ** -18                  # ceil strictness margin
CEIL_SHIFT = 0.5 - DELTA            # folded into ACT bias: ceil(x)=round(x+0.5-d)
C35_64 = 0.546875                   # 35/64 = 1/16 + 0.5 - 1/64 (exact floor shift
                                    # for values on the 1/16 grid)
N_WARMUP_MM = 8                    # PE warmup matmuls (HAM un-throttle)

_CACHE = {}


def _build():
    import concourse.bacc as bacc
    import concourse.mybir as mybir
    from concourse import tile

    f32, f16 = mybir.dt.float32, mybir.dt.float16
    A = mybir.AluOpType
    ACT_ID = mybir.ActivationFunctionType.Identity

    nc = bacc.Bacc("TRN2", target_bir_lowering=False, debug=False)

    feat_e = nc.dram_tensor("feat", [PCH, NCH, NB], f32, kind="ExternalInput").ap()
    w0_e = nc.dram_tensor("w0T", [PCH, NCH, HID], f16, kind="ExternalInput").ap()
    w1_e = nc.dram_tensor("w1T", [HID, HID], f16, kind="ExternalInput").ap()
    w2_e = nc.dram_tensor("w2T", [HID, HID], f16, kind="ExternalInput").ap()
    w3_e = nc.dram_tensor("w3T", [HID, NCH, PCH], f16, kind="ExternalInput").ap()
    b0_e = nc.dram_tensor("b0p", [HID, 1], f32, kind="ExternalInput").ap()
    b1_e = nc.dram_tensor("b1p", [HID, 1], f32, kind="ExternalInput").ap()
    b2_e = nc.dram_tensor("b2p", [HID, 1], f32, kind="ExternalInput").ap()
    b3_e = nc.dram_tensor("b3p", [PCH, NCH], f32, kind="ExternalInput").ap()
    out_e = nc.dram_tensor("out", [PCH, NCH, NB], f16, kind="ExternalOutput").ap()

    with tile.TileContext(nc) as tc:
        with (
            tc.tile_pool(name="sbuf", bufs=1) as sb,
            tc.tile_pool(name="psumh", bufs=1, space="PSUM") as psh,
            tc.tile_pool(name="psum3", bufs=4, space="PSUM") as ps3,
        ):
            # ---- loads (host pre-transposed; all partition-contiguous) ----
            feat = sb.tile([PCH, NCH, NB], f32, tag="feat")
            nc.sync.dma_start(feat[:], feat_e[:])
            w1s = sb.tile([HID, HID], f16, tag="w1")
            nc.sync.dma_start(w1s[:], w1_e[:])
            w0s = sb.tile([PCH, NCH, HID], f16, tag="w0")
            nc.sync.dma_start(w0s[:], w0_e[:])
            w2s = sb.tile([HID, HID], f16, tag="w2")
            nc.sync.dma_start(w2s[:], w2_e[:])
            w3s = sb.tile([HID, NCH, PCH], f16, tag="w3")
            nc.sync.dma_start(w3s[:], w3_e[:])
            b0s = sb.tile([HID, 1], f32, tag="b0")
            nc.sync.dma_start(b0s[:], b0_e[:])
            b1s = sb.tile([HID, 1], f32, tag="b1")
            nc.sync.dma_start(b1s[:], b1_e[:])
            b2s = sb.tile([HID, 1], f32, tag="b2")
            nc.sync.dma_start(b2s[:], b2_e[:])
            b3s = sb.tile([PCH, NCH], f32, tag="b3")
            nc.sync.dma_start(b3s[:], b3_e[:])

            # ---- scan boundary mask (1 everywhere, 0 at each chain's dummy) ----
            mask_h = sb.tile([HID, NB, SLOT], f16, tag="maskh")
            nc.gpsimd.memset(mask_h[:], 1.0)
            nc.gpsimd.memset(mask_h[:, :, NS:SLOT], 0.0)

            # ---- PE warmup: dummy matmuls so HAM un-throttles before MM0 ----
            warm = psh.tile([HID, NB * NS], f32, tag="dh0")
            warm_rhs = mask_h[:].rearrange("p j s -> p (j s)")[:, 0 : NB * NS]
            for _ in range(N_WARMUP_MM):
                nc.tensor.matmul(warm[:], w1s[:], warm_rhs, start=True, stop=True)
            # ---- input digitize: a = max(floor(16 f), 1) / 16 (exact, fp32) ----
            t1 = sb.tile([PCH, NCH, NB], f32, tag="dig1")
            nc.vector.tensor_scalar(t1[:], feat[:], 16.0, -0.5 + 2.0 ** -17, A.mult, A.add)
            t2 = sb.tile([PCH, NCH, NB], f32, tag="dig2")
            nc.vector.tensor_scalar(t2[:], t1[:], M32, -M32, A.add, A.add)
            a16 = sb.tile([PCH, NCH, NB], f16, tag="a16")
            nc.vector.tensor_scalar(a16[:], t2[:], 1.0, 1.0 / 16.0, A.max, A.mult)

            # ---- closed-form cumulative input spikes F_k = round(k*a - 35/64) ----
            # k-major layout: each per-k write is contiguous (DVE 4x mode)
            Fy = sb.tile([PCH, NS, NCH, NB], f16, tag="Fy")
            for k in range(1, NS + 1):
                nc.vector.tensor_scalar(
                    Fy[:, k - 1], a16[:], float(k), -C35_64, A.mult, A.add
                )
            F = sb.tile([PCH, NS, NCH, NB], f16, tag="F")
            nc.vector.tensor_scalar(F[:], Fy[:], M32, -M32, A.add, A.add)

            # ---- layer 0: D0 = W0 @ F, split into one accumulation group per
            # batch lane so lane A's ACT can start before lane B's matmuls.
            # rhs read in native (k, j) order (16-contiguous runs per lane).
            HBL = NB // 2
            D0L = []
            for ln in range(2):
                D = psh.tile([HID, HBL * NS], f32, tag=f"dh{ln}")
                for c in range(NCH):
                    nc.tensor.matmul(
                        D[:],
                        w0s[:, c, :],
                        F[:, :, c, ln * HBL : (ln + 1) * HBL],
                        start=(c == 0),
                        stop=(c == NCH - 1),
                    )
                D0L.append(D)

            # ---- hidden layers, two batch lanes (j 0:16 / 16:32) pipelined ----
            HB = NB // 2  # 16 images per lane

            def fire_scan(gc, lname):
                C = sb.tile([HID, HB, SLOT], f16, tag=f"C{lname}")
                nc.vector.tensor_tensor_scan(
                    C[:].rearrange("p j s -> p (j s)"),
                    gc[:].rearrange("p j s -> p (j s)"),
                    mask_h[:, 0:HB, :].rearrange("p j s -> p (j s)"),
                    0.0,
                    A.max,
                    A.mult,
                )
                return C

            def layer0_lane(lane, lname):
                # D0 lane psum is (k, j): the ACT reads it permuted to (j, k)
                # (strided psum read) and writes chain-order G contiguously.
                g = sb.tile([HID, HB, NS], f32, tag=f"g{lname}")
                din = D0L[lane][:].rearrange("p (k j) -> p j k", j=HB)
                nc.scalar.activation(g[:], din, ACT_ID, bias=b0s[:], scale=1.0)
                gc = sb.tile([HID, HB, SLOT], f16, tag=f"gc{lname}")
                nc.gpsimd.memset(gc[:, :, NS:SLOT], 0.0)
                nc.vector.tensor_scalar(
                    gc[:, :, 0:NS], g[:], M32, -M32, A.add, A.add
                )
                return fire_scan(gc, lname)

            def hidden_lane(Cin, w, bias, lname, tag):
                D = psh.tile([HID, HB * NS], f32, tag=tag)
                nc.tensor.matmul(D[:], w[:], Cin[:, :, 0:NS], start=True, stop=True)
                # psum is already (j, k) = chain order: everything contiguous
                g = sb.tile([HID, HB, NS], f32, tag=f"g{lname}")
                nc.scalar.activation(
                    g[:], D[:].rearrange("p (j k) -> p j k", k=NS),
                    ACT_ID, bias=bias[:], scale=1.0,
                )
                gc = sb.tile([HID, HB, SLOT], f16, tag=f"gc{lname}")
                nc.gpsimd.memset(gc[:, :, NS:SLOT], 0.0)
                nc.vector.tensor_scalar(
                    gc[:, :, 0:NS], g[:], M32, -M32, A.add, A.add
                )
                return fire_scan(gc, lname)

            C0 = [None, None]
            C1 = [None, None]
            C2 = [None, None]
            for ln in range(2):
                C0[ln] = layer0_lane(ln, f"0{ln}")
            for ln in range(2):
                C1[ln] = hidden_lane(C0[ln], w1s, b1s, f"1{ln}", f"dh{ln}")
            for ln in range(2):
                C2[ln] = hidden_lane(C1[ln], w2s, b2s, f"2{ln}", f"dh{ln}")

            # ---- layer 3 (output): the chase has no downstream consumer, so
            # C3_final = relu(ceil(max_k G3_k)) -- a max-reduce over k, no scan.
            Mx = sb.tile([PCH, NCH, NB], f32, tag="mx")
            U3 = sb.tile([PCH, NCH, NB], f32, tag="u3")
            for c in range(NCH):
                D3 = ps3.tile([PCH, NB * NS], f32, tag="d3")
                for ln in range(2):
                    nc.tensor.matmul(
                        D3[:, ln * HB * NS : (ln + 1) * HB * NS],
                        w3s[:, c, :],
                        C2[ln][:, :, 0:NS],
                        start=True,
                        stop=True,
                    )
                nc.vector.tensor_reduce(
                    Mx[:, c, :],
                    D3[:].rearrange("p (j k) -> p j k", k=NS),
                    mybir.AxisListType.X,
                    A.max,
                )
                # relu(G + 0.5 - delta) then round gives relu(ceil(G)) exactly
                nc.scalar.activation(
                    U3[:, c, :], Mx[:, c, :],
                    mybir.ActivationFunctionType.Relu,
                    bias=b3s[:, c : c + 1], scale=1.0,
                )
            # consume the warmup psum so DCE can't drop the matmuls (placed
            # last: ACT is strict FIFO and this op waits on the PE)
            warm_sink = sb.tile([HID, 1], f32, tag="warmsink")
            nc.scalar.activation(warm_sink[:], warm[:, 0:1], ACT_ID, bias=0.0, scale=1.0)

            o = sb.tile([PCH, NCH, NB], f16, tag="o")
            nc.vector.tensor_scalar(o[:, 0:4, :], U3[:, 0:4, :], M32, -M32, A.add, A.add)
            nc.sync.dma_start(out_e[:, 0:4, :], o[:, 0:4, :])
            nc.vector.tensor_scalar(o[:, 4:NCH, :], U3[:, 4:NCH, :], M32, -M32, A.add, A.add)
            nc.sync.dma_start(out_e[:, 4:NCH, :], o[:, 4:NCH, :])

    nc.compile()
    return nc


def _get_nc():
    if "nc" not in _CACHE:
        _CACHE["nc"] = _build()
    return _CACHE["nc"]


def _prep_in_maps(features, W0, b0, W1, b1, W2, b2, W3, b3):
    f32, f16 = np.float32, np.float16
    shift = np.float32(0.5 + DELTA)
    w0T = np.ascontiguousarray(
        W0.T.reshape(NCH, PCH, HID).transpose(1, 0, 2)
    ).astype(f16)  # [112, 7, 128]; w0T[p,c,m] = W0[m, c*112+p]
    w1T = np.ascontiguousarray(W1.T).astype(f16)
    w2T = np.ascontiguousarray(W2.T).astype(f16)
    w3T = np.ascontiguousarray(W3.T.reshape(HID, NCH, PCH)).astype(f16)
    b0p = (b0.astype(f32) - shift).reshape(HID, 1)
    b1p = (b1.astype(f32) - shift).reshape(HID, 1)
    b2p = (b2.astype(f32) - shift).reshape(HID, 1)
    b3p = np.ascontiguousarray(b3.astype(f32).reshape(NCH, PCH).T) - shift

    in_maps = []
    for i in range(NCORES):
        shard = features[i * NB : (i + 1) * NB].astype(f32)  # [32, 784]
        feat = np.ascontiguousarray(
            shard.reshape(NB, NCH, PCH).transpose(2, 1, 0)
        )  # [112, 7, 32]
        in_maps.append(
            {
                "feat": feat,
                "w0T": w0T,
                "w1T": w1T,
                "w2T": w2T,
                "w3T": w3T,
                "b0p": b0p,
                "b1p": b1p,
                "b2p": b2p,
                "b3p": np.ascontiguousarray(b3p),
            }
        )
    return in_maps


def _assemble(results):
    outs = []
    for i in range(NCORES):
        o = results[i]["out"].astype(np.float32) * np.float32(1.0 / 16.0)  # counts/16
        outs.append(o.transpose(2, 1, 0).reshape(NB, IN))  # [32, 784]
    return np.concatenate(outs, axis=0)


def kernel(features, W0, b0, W1, b1, W2, b2, W3, b3, _trace=False):
    import time
    from concourse.bass_utils import run_bass_kernel_spmd

    nc = _get_nc()
    in_maps = _prep_in_maps(features, W0, b0, W1, b1, W2, b2, W3, b3)
    # The axon-tunneled device occasionally reports a transient
    # NRT_EXEC_UNIT_UNRECOVERABLE on the first attempt after a prior process
    # exited; it recovers on retry.
    last_exc = None
    for attempt in range(3):
        try:
            res = run_bass_kernel_spmd(nc, in_maps, list(range(NCORES)), trace=_trace)
            break
        except Exception as e:  # noqa: BLE001
            last_exc = e
            time.sleep(10 * (attempt + 1))
    else:
        raise last_exc
    out = _assemble(res.results)
    if _trace:
        _CACHE["last_result"] = res
    return out


# revision 20
# speedup vs baseline: 1.1426x; 1.1426x over previous
"""Trainium2 Bass kernel for nn_AE_spikes (spiking autoencoder, 784-128-128-128-784).

Algorithm restructure (mathematically equivalent to the reference spiking net):
- Identity bin scaling (all 5 bin arrays equal) -> weights used as-is.
- Input layer digitize + integrate-and-fire has a closed form for the
  cumulative spike count: F_k = floor(k*a - 1/16), a = max(floor(16 f), 1)/16
  (m=0 and m=1 produce identical all-zero spike trains, so the clamp is exact).
- Each layer's matmul consumes the CUMULATIVE spike counts C of the previous
  layer, giving the cumulative drive D_k = W @ C_k directly (no per-step
  matmuls, no cumsum pass). One N=512 matmul per layer (K-chunked for the
  784-dim layers).
- The integrate-and-fire recurrence C_k = C_{k-1} + 1{t0 + D_k - C_{k-1} > 1}
  is computed as the max-chase C_k = max(C_{k-1}, ceil(b - 1 + D_k)), which is
  exact whenever the pre-fire potential stays <= 2 (holds for this model's
  weight scale; validated elementwise against the exact recurrence).
- ceil via the fp32 magic-round trick: round(x) = (x + 1.5*2^23) - 1.5*2^23,
  with the +0.5-delta ceil shift folded into the ACT bias.
- The whole 16-step recurrence of a layer runs as ONE DVE tensor_tensor_scan:
  state = max(Gceil_t, state) * mask_t, with 17-slot chains (16 steps + one
  masked dummy slot that resets the state to 0 between independent chains).

Sharding: pure data-parallel over the batch (256 -> 32 images per core), all
weights replicated, no collectives. Host pre-transposes weights/features so
every DMA is partition-contiguous, and reassembles the output.

Performance structure:
- fp16 matmul operands (full PE rate; counts are small integers, exact in f16).
- k-major F layout so the 16 closed-form input ops are contiguous f16 writes
  (DVE 4x mode); the layer-0 matmul reads (j,k) order via a strided rhs AP.
- Hidden layers run as two batch lanes (16 images each) pipelined across
  PE (matmul) / ACT (bias + psum read) / DVE (round + chase scan).
- Output layer needs no scan at all: its chase unrolls to
  relu(ceil(max_k G_k)) = one DVE max-reduce per 112-row chunk.
- PE warmup matmuls during the input phase so real matmuls run at 2.4 GHz.
"""
import sys

if "/opt/trn_rl_repo" not in sys.path:
    sys.path.insert(0, "/opt/trn_rl_repo")

import numpy as np

IN, HID, NS, NB = 784, 128, 16, 32  # in-dim, hidden, steps, batch per core
PCH, NCH = 112, 7                   # pixel-partition chunking: 784 = 112 * 7
SLOT = NS + 1                       # 17-slot chains (dummy slot resets scan state)
NCORES = 8
M32 = 12582912.0                    # 1.5 * 2^23: fp32 round-to-integer magic
DELTA = 2.0 ** -18                  # ceil strictness margin
CEIL_SHIFT = 0.5 - DELTA            # folded into ACT bias: ceil(x)=round(x+0.5-d)
C35_64 = 0.546875                   # 35/64 = 1/16 + 0.5 - 1/64 (exact floor shift
                                    # for values on the 1/16 grid)
N_WARMUP_MM = 8                    # PE warmup matmuls (HAM un-throttle)

_CACHE = {}


def _build():
    import concourse.bacc as bacc
    import concourse.mybir as mybir
    from concourse import tile

    f32, f16 = mybir.dt.float32, mybir.dt.float16
    A = mybir.AluOpType
    ACT_ID = mybir.ActivationFunctionType.Identity

    nc = bacc.Bacc("TRN2", target_bir_lowering=False, debug=False)

    feat_e = nc.dram_tensor("feat", [PCH, NCH, NB], f32, kind="ExternalInput").ap()
    w0_e = nc.dram_tensor("w0T", [PCH, NCH, HID], f16, kind="ExternalInput").ap()
    w1_e = nc.dram_tensor("w1T", [HID, HID], f16, kind="ExternalInput").ap()
    w2_e = nc.dram_tensor("w2T", [HID, HID], f16, kind="ExternalInput").ap()
    w3_e = nc.dram_tensor("w3T", [HID, NCH, PCH], f16, kind="ExternalInput").ap()
    b0_e = nc.dram_tensor("b0p", [HID, 1], f32, kind="ExternalInput").ap()
    b1_e = nc.dram_tensor("b1p", [HID, 1], f32, kind="ExternalInput").ap()
    b2_e = nc.dram_tensor("b2p", [HID, 1], f32, kind="ExternalInput").ap()
    b3_e = nc.dram_tensor("b3p", [PCH, NCH], f32, kind="ExternalInput").ap()
    out_e = nc.dram_tensor("out", [PCH, NCH, NB], f16, kind="ExternalOutput").ap()

    with tile.TileContext(nc) as tc:
        with (
            tc.tile_pool(name="sbuf", bufs=1) as sb,
            tc.tile_pool(name="psumh", bufs=1, space="PSUM") as psh,
            tc.tile_pool(name="psum3", bufs=5, space="PSUM") as ps3,
        ):
            # ---- loads (host pre-transposed; all partition-contiguous) ----
            feat = sb.tile([PCH, NCH, NB], f32, tag="feat")
            nc.sync.dma_start(feat[:], feat_e[:])
            w1s = sb.tile([HID, HID], f16, tag="w1")
            nc.sync.dma_start(w1s[:], w1_e[:])
            w0s = sb.tile([PCH, NCH, HID], f16, tag="w0")
            nc.sync.dma_start(w0s[:], w0_e[:])
            w2s = sb.tile([HID, HID], f16, tag="w2")
            nc.sync.dma_start(w2s[:], w2_e[:])
            w3s = sb.tile([HID, NCH, PCH], f16, tag="w3")
            nc.sync.dma_start(w3s[:], w3_e[:])
            b0s = sb.tile([HID, 1], f32, tag="b0")
            nc.sync.dma_start(b0s[:], b0_e[:])
            b1s = sb.tile([HID, 1], f32, tag="b1")
            nc.sync.dma_start(b1s[:], b1_e[:])
            b2s = sb.tile([HID, 1], f32, tag="b2")
            nc.sync.dma_start(b2s[:], b2_e[:])
            b3s = sb.tile([PCH, NCH], f32, tag="b3")
            nc.sync.dma_start(b3s[:], b3_e[:])

            # ---- scan boundary mask (1 everywhere, 0 at each chain's dummy) ----
            mask_h = sb.tile([HID, NB, SLOT], f16, tag="maskh")
            nc.gpsimd.memset(mask_h[:], 1.0)
            nc.gpsimd.memset(mask_h[:, :, NS:SLOT], 0.0)

            # ---- PE warmup: dummy matmuls so HAM un-throttles before MM0 ----
            warm = psh.tile([HID, NB * NS], f32, tag="dh0")
            warm_rhs = mask_h[:].rearrange("p j s -> p (j s)")[:, 0 : NB * NS]
            for _ in range(N_WARMUP_MM):
                nc.tensor.matmul(warm[:], w1s[:], warm_rhs, start=True, stop=True)
            # ---- input digitize: a = max(floor(16 f), 1) / 16 (exact, fp32) ----
            t1 = sb.tile([PCH, NCH, NB], f32, tag="dig1")
            nc.vector.tensor_scalar(t1[:], feat[:], 16.0, -0.5 + 2.0 ** -17, A.mult, A.add)
            t2 = sb.tile([PCH, NCH, NB], f32, tag="dig2")
            nc.vector.tensor_scalar(t2[:], t1[:], M32, -M32, A.add, A.add)
            a16 = sb.tile([PCH, NCH, NB], f16, tag="a16")
            nc.vector.tensor_scalar(a16[:], t2[:], 1.0, 1.0 / 16.0, A.max, A.mult)

            # ---- closed-form cumulative input spikes F_k = round(k*a - 35/64) ----
            # k-major layout: each per-k write is contiguous (DVE 4x mode)
            Fy = sb.tile([PCH, NS, NCH, NB], f16, tag="Fy")
            for k in range(1, NS + 1):
                nc.vector.tensor_scalar(
                    Fy[:, k - 1], a16[:], float(k), -C35_64, A.mult, A.add
                )
            F = sb.tile([PCH, NS, NCH, NB], f16, tag="F")
            nc.vector.tensor_scalar(F[:], Fy[:], M32, -M32, A.add, A.add)

            # ---- layer 0: D0 = W0 @ F, split into one accumulation group per
            # batch lane so lane A's ACT can start before lane B's matmuls.
            # rhs read in native (k, j) order (16-contiguous runs per lane).
            HBL = NB // 2
            D0L = []
            for ln in range(2):
                D = psh.tile([HID, HBL * NS], f32, tag=f"dh{ln}")
                for c in range(NCH):
                    nc.tensor.matmul(
                        D[:],
                        w0s[:, c, :],
                        F[:, :, c, ln * HBL : (ln + 1) * HBL],
                        start=(c == 0),
                        stop=(c == NCH - 1),
                    )
                D0L.append(D)

            # ---- hidden layers, two batch lanes (j 0:16 / 16:32) pipelined ----
            HB = NB // 2  # 16 images per lane

            def fire_scan(gc, lname):
                C = sb.tile([HID, HB, SLOT], f16, tag=f"C{lname}")
                nc.vector.tensor_tensor_scan(
                    C[:].rearrange("p j s -> p (j s)"),
                    gc[:].rearrange("p j s -> p (j s)"),
                    mask_h[:, 0:HB, :].rearrange("p j s -> p (j s)"),
                    0.0,
                    A.max,
                    A.mult,
                )
                return C

            def layer0_lane(lane, lname):
                # D0 lane psum is (k, j): the ACT reads it permuted to (j, k)
                # (strided psum read) and writes chain-order G contiguously.
                g = sb.tile([HID, HB, NS], f32, tag=f"g{lname}")
                din = D0L[lane][:].rearrange("p (k j) -> p j k", j=HB)
                nc.scalar.activation(g[:], din, ACT_ID, bias=b0s[:], scale=1.0)
                gc = sb.tile([HID, HB, SLOT], f16, tag=f"gc{lname}")
                nc.gpsimd.memset(gc[:, :, NS:SLOT], 0.0)
                nc.vector.tensor_scalar(
                    gc[:, :, 0:NS], g[:], M32, -M32, A.add, A.add
                )
                return fire_scan(gc, lname)

            def hidden_lane(Cin, w, bias, lname, tag):
                D = psh.tile([HID, HB * NS], f32, tag=tag)
                nc.tensor.matmul(D[:], w[:], Cin[:, :, 0:NS], start=True, stop=True)
                # psum is already (j, k) = chain order: everything contiguous
                g = sb.tile([HID, HB, NS], f32, tag=f"g{lname}")
                nc.scalar.activation(
                    g[:], D[:].rearrange("p (j k) -> p j k", k=NS),
                    ACT_ID, bias=bias[:], scale=1.0,
                )
                gc = sb.tile([HID, HB, SLOT], f16, tag=f"gc{lname}")
                nc.gpsimd.memset(gc[:, :, NS:SLOT], 0.0)
                nc.vector.tensor_scalar(
                    gc[:, :, 0:NS], g[:], M32, -M32, A.add, A.add
                )
                return fire_scan(gc, lname)

            C0 = [None, None]
            C1 = [None, None]
            C2 = [None, None]
            for ln in range(2):
                C0[ln] = layer0_lane(ln, f"0{ln}")
            for ln in range(2):
                C1[ln] = hidden_lane(C0[ln], w1s, b1s, f"1{ln}", f"dh{ln}")
            for ln in range(2):
                C2[ln] = hidden_lane(C1[ln], w2s, b2s, f"2{ln}", f"dh{ln}")

            # ---- layer 3 (output): the chase has no downstream consumer, so
            # C3_final = relu(ceil(max_k G3_k)) -- a max-reduce over k, no scan.
            Mx = sb.tile([PCH, NCH, NB], f32, tag="mx")
            U3 = sb.tile([PCH, NCH, NB], f32, tag="u3")
            for c in range(NCH):
                D3 = ps3.tile([PCH, NB * NS], f32, tag="d3")
                for ln in range(2):
                    nc.tensor.matmul(
                        D3[:, ln * HB * NS : (ln + 1) * HB * NS],
                        w3s[:, c, :],
                        C2[ln][:, :, 0:NS],
                        start=True,
                        stop=True,
                    )
                nc.vector.tensor_reduce(
                    Mx[:, c, :],
                    D3[:].rearrange("p (j k) -> p j k", k=NS),
                    mybir.AxisListType.X,
                    A.max,
                )
                # relu(G + 0.5 - delta) then round gives relu(ceil(G)) exactly
                nc.scalar.activation(
                    U3[:, c, :], Mx[:, c, :],
                    mybir.ActivationFunctionType.Relu,
                    bias=b3s[:, c : c + 1], scale=1.0,
                )
            # consume the warmup psum so DCE can't drop the matmuls (placed
            # last: ACT is strict FIFO and this op waits on the PE)
            warm_sink = sb.tile([HID, 1], f32, tag="warmsink")
            nc.scalar.activation(warm_sink[:], warm[:, 0:1], ACT_ID, bias=0.0, scale=1.0)

            o = sb.tile([PCH, NCH, NB], f16, tag="o")
            nc.vector.tensor_scalar(o[:, 0:4, :], U3[:, 0:4, :], M32, -M32, A.add, A.add)
            nc.sync.dma_start(out_e[:, 0:4, :], o[:, 0:4, :])
            nc.vector.tensor_scalar(o[:, 4:NCH, :], U3[:, 4:NCH, :], M32, -M32, A.add, A.add)
            nc.sync.dma_start(out_e[:, 4:NCH, :], o[:, 4:NCH, :])

    nc.compile()
    return nc


def _get_nc():
    if "nc" not in _CACHE:
        _CACHE["nc"] = _build()
    return _CACHE["nc"]


def _prep_in_maps(features, W0, b0, W1, b1, W2, b2, W3, b3):
    f32, f16 = np.float32, np.float16
    shift = np.float32(0.5 + DELTA)
    w0T = np.ascontiguousarray(
        W0.T.reshape(NCH, PCH, HID).transpose(1, 0, 2)
    ).astype(f16)  # [112, 7, 128]; w0T[p,c,m] = W0[m, c*112+p]
    w1T = np.ascontiguousarray(W1.T).astype(f16)
    w2T = np.ascontiguousarray(W2.T).astype(f16)
    w3T = np.ascontiguousarray(W3.T.reshape(HID, NCH, PCH)).astype(f16)
    b0p = (b0.astype(f32) - shift).reshape(HID, 1)
    b1p = (b1.astype(f32) - shift).reshape(HID, 1)
    b2p = (b2.astype(f32) - shift).reshape(HID, 1)
    b3p = np.ascontiguousarray(b3.astype(f32).reshape(NCH, PCH).T) - shift

    in_maps = []
    for i in range(NCORES):
        shard = features[i * NB : (i + 1) * NB].astype(f32)  # [32, 784]
        feat = np.ascontiguousarray(
            shard.reshape(NB, NCH, PCH).transpose(2, 1, 0)
        )  # [112, 7, 32]
        in_maps.append(
            {
                "feat": feat,
                "w0T": w0T,
                "w1T": w1T,
                "w2T": w2T,
                "w3T": w3T,
                "b0p": b0p,
                "b1p": b1p,
                "b2p": b2p,
                "b3p": np.ascontiguousarray(b3p),
            }
        )
    return in_maps


def _assemble(results):
    outs = []
    for i in range(NCORES):
        o = results[i]["out"].astype(np.float32) * np.float32(1.0 / 16.0)  # counts/16
        outs.append(o.transpose(2, 1, 0).reshape(NB, IN))  # [32, 784]
    return np.concatenate(outs, axis=0)


def kernel(features, W0, b0, W1, b1, W2, b2, W3, b3, _trace=False):
    import time
    from concourse.bass_utils import run_bass_kernel_spmd

    nc = _get_nc()
    in_maps = _prep_in_maps(features, W0, b0, W1, b1, W2, b2, W3, b3)
    # The axon-tunneled device occasionally reports a transient
    # NRT_EXEC_UNIT_UNRECOVERABLE on the first attempt after a prior process
    # exited; it recovers on retry.
    last_exc = None
    for attempt in range(3):
        try:
            res = run_bass_kernel_spmd(nc, in_maps, list(range(NCORES)), trace=_trace)
            break
        except Exception as e:  # noqa: BLE001
            last_exc = e
            time.sleep(10 * (attempt + 1))
    else:
        raise last_exc
    out = _assemble(res.results)
    if _trace:
        _CACHE["last_result"] = res
    return out
